# revision 4
# baseline (speedup 1.0000x reference)
"""AFNO2D Trainium2 kernel (8 NeuronCores, channel-sharded, zero collectives).

Each core processes one 96-channel block (FFT is per-channel; the MLP is
block-diagonal with exactly 8 blocks of 96 -> core i owns block i end-to-end).

Per-core pipeline (all matmuls bf16, fp32 PSUM):
  S1  rfft over W:   lhsT=F1 [w,128]=[cos(65)|-sin(63)], rhs=x [w,(h,c)]
                     -> psum [kwstack,(h,c)], evict -> t1 [kw, c, h] bf16
  T1  DMA-xbar transpose: t1 -> t2 [h, c, kwstack]
  S2  DFT over H (data stationary): per kw: lhsT=t2[:, :, kw*] [h, c],
                     rhs=F2a/F2b [h, 256] -> psum [c, (khr|khi)] -> specw
  MLP1 (rhs mode):   lhsT=w1*, rhs=spec cols -> psum [co, pos],
                     ACT relu+b1 evict -> o1w [co, ri, kw, kh]
  MLP2 (data stationary): per kw: lhsT=o1w slices [co, kh], rhs=[w2r|w2i] etc
                     + K=1 ones x [b2r|b2i] bias matmul -> psum [kh, (cr|ci)]
  softshrink:        relu(v-l) + min(v+l, 0)  (ACT + DVE + gpsimd add)
  S4  iDFT over H (rhs mode): lhsT=Gc/Gs/-Gs, rhs=o2 cols -> ur/ui [h,(kw,c)]
                     evict -> ubuf [h, c, kwstack] (ui bins 0,64 dropped)
  T3  DMA-xbar transpose: ubuf -> s5rhs [kwstack, c, h]
  S5  irfft over W:  lhsT=Ainv [kstack, w], rhs=s5rhs cols (c,h)
                     -> psum [w,(c,h)], DVE add residual -> out [w, h, c] f32
"""
import numpy as np
import ml_dtypes

B, H, W, C = 4, 128, 128, 768
NB, BL = 8, 96
WF = 65
LAMBD = 0.01
BF16 = ml_dtypes.bfloat16

_CACHE = {}


def _make_consts():
    w = np.arange(W, dtype=np.float64)[:, None]
    k = np.arange(WF, dtype=np.float64)[None, :]
    th = 2 * np.pi * w * k / W
    s = 1.0 / np.sqrt(W)
    f1 = np.concatenate([np.cos(th) * s, -np.sin(th[:, 1:64]) * s], axis=1)

    h = np.arange(H, dtype=np.float64)[:, None]
    kh = np.arange(H, dtype=np.float64)[None, :]
    th2 = 2 * np.pi * h * kh / H
    c2 = np.cos(th2) / np.sqrt(H)
    s2 = np.sin(th2) / np.sqrt(H)
    f2a = np.concatenate([c2, -s2], axis=1)   # rhs when lhsT = t_r
    f2b = np.concatenate([s2, c2], axis=1)    # rhs when lhsT = t_i

    gc = (np.cos(th2) / np.sqrt(H)).T         # [kh, h] (symmetric, .T for clarity)
    gs = (np.sin(th2) / np.sqrt(H)).T

    kk = np.arange(WF, dtype=np.float64)[:, None]
    ww = np.arange(W, dtype=np.float64)[None, :]
    th3 = 2 * np.pi * kk * ww / W
    beta = np.full((WF, 1), 2.0); beta[0] = 1.0; beta[64] = 1.0
    ac = beta * np.cos(th3) / np.sqrt(W)
    asn = -2.0 * np.sin(th3[1:64]) / np.sqrt(W)
    ainv = np.concatenate([ac, asn], axis=0)

    cast = lambda a: np.ascontiguousarray(a).astype(BF16)
    return dict(f1=cast(f1), f2a=cast(f2a), f2b=cast(f2b),
                gc=cast(gc), gs=cast(gs), gsn=cast(-gs), ainv=cast(ainv))


def _groups():
    gs = [list(range(i, i + 8)) for i in range(0, 64, 8)]
    gs.append([64])
    return gs


def _build():
    from contextlib import ExitStack
    from concourse import bacc, mybir, tile

    dt = mybir.dt
    nc = bacc.Bacc("TRN2", target_bir_lowering=False, debug=False, num_devices=8)

    x_d = nc.dram_tensor("x", [B, H, W, BL], dt.bfloat16, kind="ExternalInput")
    f1_d = nc.dram_tensor("f1", [128, 128], dt.bfloat16, kind="ExternalInput")
    f2a_d = nc.dram_tensor("f2a", [128, 256], dt.bfloat16, kind="ExternalInput")
    f2b_d = nc.dram_tensor("f2b", [128, 256], dt.bfloat16, kind="ExternalInput")
    gc_d = nc.dram_tensor("gc", [128, 128], dt.bfloat16, kind="ExternalInput")
    gs_d = nc.dram_tensor("gs", [128, 128], dt.bfloat16, kind="ExternalInput")
    gsn_d = nc.dram_tensor("gsn", [128, 128], dt.bfloat16, kind="ExternalInput")
    ainv_d = nc.dram_tensor("ainv", [128, 128], dt.bfloat16, kind="ExternalInput")
    w1r_d = nc.dram_tensor("w1r", [BL, BL], dt.bfloat16, kind="ExternalInput")
    w1i_d = nc.dram_tensor("w1i", [BL, BL], dt.bfloat16, kind="ExternalInput")
    w1in_d = nc.dram_tensor("w1in", [BL, BL], dt.bfloat16, kind="ExternalInput")
    w2a_d = nc.dram_tensor("w2a", [BL, 192], dt.bfloat16, kind="ExternalInput")
    w2b_d = nc.dram_tensor("w2b", [BL, 192], dt.bfloat16, kind="ExternalInput")
    b1r_d = nc.dram_tensor("b1r", [BL, 1], dt.float32, kind="ExternalInput")
    b1i_d = nc.dram_tensor("b1i", [BL, 1], dt.float32, kind="ExternalInput")
    b2c_d = nc.dram_tensor("b2c", [1, 192], dt.bfloat16, kind="ExternalInput")
    ones_d = nc.dram_tensor("onesrow", [1, 128], dt.bfloat16, kind="ExternalInput")
    out_d = nc.dram_tensor("out", [B, H, W, BL], dt.float32, kind="ExternalOutput")

    Relu = mybir.ActivationFunctionType.Relu
    Copy = mybir.ActivationFunctionType.Copy
    ADD = mybir.AluOpType.add
    MIN = mybir.AluOpType.min

    with tile.TileContext(nc) as tc, ExitStack() as ctx:
        cp = ctx.enter_context(tc.tile_pool(name="const", bufs=1))
        xp = ctx.enter_context(tc.tile_pool(name="xb", bufs=2))
        t1p = ctx.enter_context(tc.tile_pool(name="t1", bufs=1))
        t2p = ctx.enter_context(tc.tile_pool(name="t2", bufs=1))
        sw = ctx.enter_context(tc.tile_pool(name="specw", bufs=2))
        o1p = ctx.enter_context(tc.tile_pool(name="o1w", bufs=2))
        o2p = ctx.enter_context(tc.tile_pool(name="o2w", bufs=2))
        tap = ctx.enter_context(tc.tile_pool(name="tmpa", bufs=2))
        tbp = ctx.enter_context(tc.tile_pool(name="tmpb", bufs=2))
        up = ctx.enter_context(tc.tile_pool(name="ubuf", bufs=1))
        s5p = ctx.enter_context(tc.tile_pool(name="s5rhs", bufs=1))
        ocp = ctx.enter_context(tc.tile_pool(name="outc", bufs=3))
        psm = ctx.enter_context(tc.tile_pool(name="psmain", bufs=3, space="PSUM"))
        ps2p = ctx.enter_context(tc.tile_pool(name="ps2", bufs=2, space="PSUM"))
        pm2p = ctx.enter_context(tc.tile_pool(name="psm2", bufs=2, space="PSUM"))

        def cload(dram, shape, dtype=dt.bfloat16):
            t = cp.tile(shape, dtype, tag=f"c_{dram.name}")
            nc.sync.dma_start(t[:], dram[:])
            return t

        f1 = cload(f1_d, [128, 128]); f2a = cload(f2a_d, [128, 256])
        f2b = cload(f2b_d, [128, 256]); gc = cload(gc_d, [128, 128])
        gs = cload(gs_d, [128, 128]); gsn = cload(gsn_d, [128, 128])
        ainv = cload(ainv_d, [128, 128]); w1r = cload(w1r_d, [BL, BL])
        w1i = cload(w1i_d, [BL, BL]); w1in = cload(w1in_d, [BL, BL])
        w2a = cload(w2a_d, [BL, 192]); w2b = cload(w2b_d, [BL, 192])
        b2c = cload(b2c_d, [1, 192]); ones = cload(ones_d, [1, 128])
        b1r = cload(b1r_d, [BL, 1], dt.float32)
        b1i = cload(b1i_d, [BL, 1], dt.float32)
        lamneg = cp.tile([128, 1], dt.float32, tag="c_lamneg")
        nc.gpsimd.memset(lamneg[:], -LAMBD)

        GROUPS = _groups()

        for b in range(B):
            xb = xp.tile([128, 128, BL], dt.bfloat16, tag="xb")  # [w, h, c]
            nc.sync.dma_start(xb[:], x_d[b].rearrange("h w c -> w h c"))

            # ---- S1
            t1 = t1p.tile([128, BL, 128], dt.bfloat16, tag="t1")  # [kw, c, h]
            for hc in range(0, 128, 4):
                ps = psm.tile([128, 4, BL], dt.float32, tag="ps")
                nc.tensor.matmul(ps[:], f1[:], xb[:, hc:hc + 4, :],
                                 start=True, stop=True)
                dst = t1[:, :, hc:hc + 4].rearrange("k c h -> k h c")
                nc.scalar.activation(dst, ps[:], Copy)

            # ---- T1
            t2 = t2p.tile([128, BL, 128], dt.bfloat16, tag="t2")  # [h, c, kwstack]
            nc.scalar.dma_start_transpose(t2[:], t1[:])

            # ---- per kw-group middle section
            ub = up.tile([128, BL, 128], dt.bfloat16, tag="ub")  # [h, c, kwstack]
            for grp in GROUPS:
                g0, gl = grp[0], len(grp)
                spec = sw.tile([BL, 8, 2, 128], dt.bfloat16, tag="spec")
                # S2: two kw per psum tile
                for j0 in range(0, gl, 2):
                    jl = min(2, gl - j0)
                    ps2 = ps2p.tile([BL, 2, 2, 128], dt.float32, tag="ps2")
                    for j in range(j0, j0 + jl):
                        kw = g0 + j
                        edge = kw in (0, 64)
                        nc.tensor.matmul(ps2[:, j - j0, :, :], t2[:, :, kw],
                                         f2a[:], start=True, stop=edge)
                        if not edge:
                            nc.tensor.matmul(ps2[:, j - j0, :, :],
                                             t2[:, :, 64 + kw], f2b[:],
                                             start=False, stop=True)
                    nc.vector.tensor_copy(spec[:, j0:j0 + jl, :, :],
                                          ps2[:, 0:jl, :, :])
                # MLP1 over this window
                o1 = o1p.tile([BL, 2, 8, 128], dt.bfloat16, tag="o1")
                for c0 in range(0, gl, 4):
                    cl = min(4, gl - c0)
                    xr = spec[:, c0:c0 + cl, 0, :]
                    xi = spec[:, c0:c0 + cl, 1, :]
                    pr = psm.tile([BL, 4, 128], dt.float32, tag="ps")
                    nc.tensor.matmul(pr[:, 0:cl, :], w1r[:], xr, start=True, stop=False)
                    nc.tensor.matmul(pr[:, 0:cl, :], w1in[:], xi, start=False, stop=True)
                    nc.scalar.activation(o1[:, 0, c0:c0 + cl, :], pr[:, 0:cl, :],
                                         Relu, bias=b1r[:])
                    pi = psm.tile([BL, 4, 128], dt.float32, tag="ps")
                    nc.tensor.matmul(pi[:, 0:cl, :], w1i[:], xr, start=True, stop=False)
                    nc.tensor.matmul(pi[:, 0:cl, :], w1r[:], xi, start=False, stop=True)
                    nc.scalar.activation(o1[:, 1, c0:c0 + cl, :], pi[:, 0:cl, :],
                                         Relu, bias=b1i[:])
                # MLP2 + bias + softshrink
                o2 = o2p.tile([128, 2, 8, BL], dt.bfloat16, tag="o2")  # [kh, ri, kw, c]
                for j0 in range(0, gl, 2):
                    jl = min(2, gl - j0)
                    pm = pm2p.tile([128, 2, 2, BL], dt.float32, tag="pm2")
                    for j in range(j0, j0 + jl):
                        nc.tensor.matmul(pm[:, j - j0, :, :], o1[:, 0, j, :],
                                         w2a[:], start=True, stop=False)
                        nc.tensor.matmul(pm[:, j - j0, :, :], o1[:, 1, j, :],
                                         w2b[:], start=False, stop=False)
                        nc.tensor.matmul(pm[:, j - j0, :, :], ones[:], b2c[:],
                                         start=False, stop=True)
                    # softshrink(v) = relu(v - l) + min(v + l, 0)
                    ta = tap.tile([128, 2, 2, BL], dt.bfloat16, tag="ta")
                    tb = tbp.tile([128, 2, 2, BL], dt.bfloat16, tag="tb")
                    nc.scalar.activation(ta[:, 0:jl], pm[:, 0:jl], Relu,
                                         bias=lamneg[:])
                    nc.vector.tensor_scalar(tb[:, 0:jl], pm[:, 0:jl],
                                            LAMBD, 0.0, ADD, MIN)
                    dst = o2[:, :, j0:j0 + jl, :].rearrange("k r w c -> k w r c")
                    nc.gpsimd.tensor_tensor(dst, ta[:, 0:jl], tb[:, 0:jl], ADD)
                # S4 over this window
                for c0 in range(0, gl, 4):
                    cl = min(4, gl - c0)
                    o2r = o2[:, 0, c0:c0 + cl, :]
                    o2i = o2[:, 1, c0:c0 + cl, :]
                    pu = psm.tile([128, 4, BL], dt.float32, tag="ps")
                    nc.tensor.matmul(pu[:, 0:cl, :], gc[:], o2r, start=True, stop=False)
                    nc.tensor.matmul(pu[:, 0:cl, :], gsn[:], o2i, start=False, stop=True)
                    dst = ub[:, :, g0 + c0:g0 + c0 + cl].rearrange("h c k -> h k c")
                    nc.scalar.activation(dst, pu[:, 0:cl, :], Copy)
                    if g0 + c0 == 64:
                        continue  # kw=64 needs no ui
                    pv = psm.tile([128, 4, BL], dt.float32, tag="ps")
                    nc.tensor.matmul(pv[:, 0:cl, :], gs[:], o2r, start=True, stop=False)
                    nc.tensor.matmul(pv[:, 0:cl, :], gc[:], o2i, start=False, stop=True)
                    lo = 1 if g0 + c0 == 0 else 0  # drop kw=0 column
                    kwlo = g0 + c0 + lo
                    dst = ub[:, :, 64 + kwlo:64 + g0 + c0 + cl].rearrange("h c k -> h k c")
                    nc.vector.tensor_copy(dst, pv[:, lo:cl, :])

            # ---- T3
            s5r = s5p.tile([128, BL, 128], dt.bfloat16, tag="s5r")  # [kstack, c, h]
            nc.scalar.dma_start_transpose(s5r[:], ub[:])

            # ---- S5 + residual + out
            for hh in range(0, 128, 4):
                ps5 = psm.tile([128, BL, 4], dt.float32, tag="ps")
                nc.tensor.matmul(ps5[:], ainv[:], s5r[:, :, hh:hh + 4],
                                 start=True, stop=True)
                oc = ocp.tile([128, 4, BL], dt.float32, tag="oc")  # [w, h, c]
                nc.vector.tensor_tensor(
                    oc[:].rearrange("w h c -> w c h"), ps5[:],
                    xb[:, hh:hh + 4, :].rearrange("w h c -> w c h"), ADD)
                nc.sync.dma_start(
                    out_d[b, hh:hh + 4, :, :].rearrange("h w c -> w h c"), oc[:])

    nc.compile()
    return nc


def get_nc():
    if "nc" not in _CACHE:
        _CACHE["nc"] = _build()
    return _CACHE["nc"]


def make_in_maps(x, w1, b1, w2, b2):
    consts = _make_consts()
    f32 = np.float32
    in_maps = []
    for i in range(NB):
        sl = slice(BL * i, BL * (i + 1))
        m = dict(consts)
        m["x"] = np.ascontiguousarray(x[..., sl]).astype(BF16)
        m["w1r"] = np.ascontiguousarray(w1[0, i]).astype(BF16)
        m["w1i"] = np.ascontiguousarray(w1[1, i]).astype(BF16)
        m["w1in"] = np.ascontiguousarray(-w1[1, i]).astype(BF16)
        m["w2a"] = np.concatenate([w2[0, i], w2[1, i]], axis=1).astype(BF16)
        m["w2b"] = np.concatenate([-w2[1, i], w2[0, i]], axis=1).astype(BF16)
        m["b1r"] = np.ascontiguousarray(b1[0, i][:, None]).astype(f32)
        m["b1i"] = np.ascontiguousarray(b1[1, i][:, None]).astype(f32)
        m["b2c"] = np.concatenate([b2[0, i], b2[1, i]])[None, :].astype(BF16)
        m["onesrow"] = np.ones((1, 128), BF16)
        in_maps.append(m)
    return in_maps


def kernel(x, w1, b1, w2, b2):
    from concourse.bass_utils import run_bass_kernel_spmd
    nc = get_nc()
    in_maps = make_in_maps(np.asarray(x), np.asarray(w1), np.asarray(b1),
                           np.asarray(w2), np.asarray(b2))
    res = run_bass_kernel_spmd(nc, in_maps, core_ids=list(range(NB)))
    out = np.concatenate([res.results[i]["out"] for i in range(NB)], axis=-1)
    return out.astype(np.float32)
